# revision 6
# baseline (speedup 1.0000x reference)
"""Trainium2 Bass kernel for nn_Attention_13348758356565.

Dense transformer attention block (B=16, N=1024 tokens, DIM=1024, 16 heads x 64)
with axial rotary embeddings, data-parallel over batch across 8 NeuronCores
(2 batches per core). All matmuls on TensorE at full rate: QKV projection in
float32r (~1.5e-4 matmul precision), attention QK^T in float32r with 2-head
row-group packing (K=64), P*V and output projection in bf16. Softmax without
max-subtraction (scores are O(1)); denominators come free from an appended
ones-column in the PV stationary operand; normalization uses a GPSIMD
partition-broadcast of a DVE fast reciprocal.
"""

import os
import sys

sys.path.insert(0, "/opt/trn_rl_repo")

import dataclasses
import numpy as np

import concourse.bacc as bacc
import concourse.mybir as mybir
import concourse.tile as tile
from concourse import bass_utils

F32 = mybir.dt.float32
F32R = mybir.dt.float32r
BF16 = mybir.dt.bfloat16
EXP = mybir.ActivationFunctionType.Exp

B, HF, WF = 16, 32, 32
DIM, NH, HD = 1024, 16, 64
N = HF * WF          # 1024 tokens
NCORES = 8
BPC = B // NCORES    # 2 batches per core
ROT = HD // 2        # 32 rotary dims per head
SCALE = 1.0 / np.sqrt(HD)

last_exec_time_ns = None


def _round13(x):
    """Round fp32 mantissa to 13 bits (safe operand form for f32r matmuls)."""
    xi = np.ascontiguousarray(x, np.float32).view(np.uint32)
    xi = ((xi.astype(np.uint64) + (1 << 9)) >> 10 << 10).astype(np.uint32)
    return xi.view(np.float32)


def _bcast_mid(ap, count):
    """Insert a step-0 (broadcast) middle dim into a [P, C] AP -> [P, count, C]."""
    return dataclasses.replace(ap, ap=[ap.ap[0], [0, count], ap.ap[1]])


def _freq_tables():
    d = HD // 4
    base = (np.linspace(1.0, (HF * WF) / 2.0, d // 2, dtype=np.float64) * np.pi)
    posH = np.linspace(-1.0, 1.0, HF)
    posW = np.linspace(-1.0, 1.0, WF)
    fH = np.repeat(posH[:, None] * base[None, :], 2, axis=-1)   # [H, 16]
    fW = np.repeat(posW[:, None] * base[None, :], 2, axis=-1)   # [W, 16]
    fH = np.broadcast_to(fH[:, None, :], (HF, WF, d))
    fW = np.broadcast_to(fW[None, :, :], (HF, WF, d))
    freqs = np.concatenate([fH, fW], axis=-1).reshape(N, ROT)
    # freqs[:, 2i] == freqs[:, 2i+1]; keep one per pair -> [N, 16]
    half = freqs[:, 0::2].astype(np.float32)
    return np.cos(half).astype(np.float32), np.sin(half).astype(np.float32)


def _build():
    nc = bacc.Bacc("TRN2", target_bir_lowering=False, debug=False)

    xT_d = nc.dram_tensor("xT", [BPC, DIM, N], F32R, kind="ExternalInput")
    wqkvT_d = nc.dram_tensor("wqkvT", [DIM, 3 * DIM], F32R, kind="ExternalInput")
    wprojT_d = nc.dram_tensor("wprojT", [DIM, DIM], BF16, kind="ExternalInput")
    bproj_d = nc.dram_tensor("bproj", [1, DIM], BF16, kind="ExternalInput")
    cosh_d = nc.dram_tensor("cosh", [N, 16], F32, kind="ExternalInput")
    sinh_d = nc.dram_tensor("sinh", [N, 16], F32, kind="ExternalInput")
    ident_d = nc.dram_tensor("ident", [128, 128], F32R, kind="ExternalInput")
    ones_d = nc.dram_tensor("ones", [1, 128], BF16, kind="ExternalInput")
    y_d = nc.dram_tensor("y", [BPC, N, DIM], F32, kind="ExternalOutput")

    NT = N // 128            # 8 token tiles
    ND = DIM // 128          # 8 contraction tiles
    HP = NH // 2             # 8 head pairs

    with tile.TileContext(nc) as tc:
        with (
            tc.tile_pool(name="sb", bufs=1) as sb,
            tc.tile_pool(name="ps", bufs=1, space="PSUM") as ps,
        ):
            # ---- constants ----
            ident = sb.tile([128, 128], F32R, name="ident")
            nc.sync.dma_start(ident[:], ident_d.ap())
            ones_r = sb.tile([1, 128], BF16, name="ones_r")
            nc.sync.dma_start(ones_r[:], ones_d.ap())
            bproj = sb.tile([1, DIM], BF16, name="bproj")
            nc.sync.dma_start(bproj[:], bproj_d.ap())
            cosh = sb.tile([128, NT * 16], F32, name="cosh")
            sinh = sb.tile([128, NT * 16], F32, name="sinh")
            nc.sync.dma_start(
                cosh[:].rearrange("p (t c) -> p t c", c=16),
                cosh_d.ap().rearrange("(t p) c -> p t c", p=128),
            )
            nc.sync.dma_start(
                sinh[:].rearrange("p (t c) -> p t c", c=16),
                sinh_d.ap().rearrange("(t p) c -> p t c", p=128),
            )
            wprojT = [sb.tile([128, DIM], BF16, name=f"wprojT{d}") for d in range(ND)]
            for d in range(ND):
                nc.sync.dma_start(wprojT[d][:], wprojT_d.ap()[d * 128:(d + 1) * 128, :])

            # persistent per-b buffers (tags reused across b)
            qT = [sb.tile([128, N], F32R, name=f"qT{j}", tag=f"qT{j}") for j in range(ND)]
            kT = [sb.tile([128, N], F32R, name=f"kT{j}", tag=f"kT{j}") for j in range(ND)]
            vsb = [sb.tile([128, NH * (HD + 1)], BF16, name=f"v{t}", tag=f"v{t}")
                   for t in range(NT)]

            for b in range(BPC):
                # ================= Phase 1: QKV + rotary + transposes ======
                xT = []
                for d in range(ND):
                    xt = sb.tile([128, N], F32R, name=f"xT_b{b}_{d}", tag=f"xT{d}")
                    nc.sync.dma_start(xt[:], xT_d.ap()[b, d * 128:(d + 1) * 128, :])
                    xT.append(xt)

                for t in range(NT):
                    # ones column for the PV denominator trick
                    nc.vector.memset(
                        vsb[t][:].rearrange("p (h c) -> p h c", c=HD + 1)[:, :, HD:],
                        1.0,
                    )

                for jc in range(6):  # chunks of 512 over j = [q(2) k(2) v(2)]
                    wq = [sb.tile([128, 512], F32R, name=f"wq_b{b}_{jc}_{d}",
                                  tag=f"wq{d}", bufs=2)
                          for d in range(ND)]
                    for d in range(ND):
                        nc.sync.dma_start(
                            wq[d][:],
                            wqkvT_d.ap()[d * 128:(d + 1) * 128, jc * 512:(jc + 1) * 512],
                        )
                    sect = jc // 2  # 0=q 1=k 2=v
                    qn_tiles = []
                    for t in range(NT):
                        pq = ps.tile([128, 512], F32, name=f"pqkv_b{b}_{jc}_{t}",
                                     tag="mm512", bufs=2)
                        for d in range(ND):
                            nc.tensor.matmul(
                                pq[:],
                                xT[d][:, t * 128:(t + 1) * 128],
                                wq[d][:],
                                start=(d == 0), stop=(d == ND - 1),
                            )
                        if sect < 2:
                            # rotary on heads h0..h0+7 (hd pairs in first 32 dims)
                            qn = sb.tile([128, 512], F32R, name=f"qn_b{b}_{jc}_{t}",
                                         tag="work2k", bufs=10)
                            pr = pq[:].rearrange("p (h i u) -> p h i u", i=32, u=2)
                            on = qn[:].rearrange("p (h i u) -> p h i u", i=32, u=2)
                            ev, od = pr[:, :, 0:16, 0], pr[:, :, 0:16, 1]
                            oev, ood = on[:, :, 0:16, 0], on[:, :, 0:16, 1]
                            cb = _bcast_mid(cosh[:, t * 16:(t + 1) * 16], 8)
                            sbb = _bcast_mid(sinh[:, t * 16:(t + 1) * 16], 8)
                            t1 = sb.tile([128, 8, 16], F32, name=f"t1_b{b}_{jc}_{t}",
                                         tag="rtmp", bufs=2)
                            t2 = sb.tile([128, 8, 16], F32, name=f"t2_b{b}_{jc}_{t}",
                                         tag="rtmp2", bufs=2)
                            mul = mybir.AluOpType.mult
                            sub = mybir.AluOpType.subtract
                            add = mybir.AluOpType.add
                            nc.vector.tensor_tensor(t1[:], ev, cb, mul)
                            nc.vector.tensor_tensor(t2[:], od, sbb, mul)
                            nc.vector.tensor_tensor(oev, t1[:], t2[:], sub)
                            nc.vector.tensor_tensor(t1[:], od, cb, mul)
                            nc.vector.tensor_tensor(t2[:], ev, sbb, mul)
                            nc.vector.tensor_tensor(ood, t1[:], t2[:], add)
                            # pass-through half (hd 32..63)
                            nc.vector.tensor_copy(on[:, :, 16:32, :], pr[:, :, 16:32, :])
                            qn_tiles.append(qn)
                        else:
                            # v: natural layout, strided into 65-wide head slots
                            h0 = (jc - 4) * 8
                            nc.vector.tensor_copy(
                                vsb[t][:].rearrange("p (h c) -> p h c", c=HD + 1)
                                [:, h0:h0 + 8, 0:HD],
                                pq[:].rearrange("p (h c) -> p h c", c=HD),
                            )
                    if sect < 2:
                        # transpose qn blocks into qT/kT [j, n] layout
                        dst = qT if sect == 0 else kT
                        for jt in range(4):  # j-tiles inside this 512 chunk
                            jg = (jc % 2) * 4 + jt
                            for g in range(2):  # groups of 4 token tiles
                                tp = ps.tile([128, 512], F32R,
                                             name=f"tp_b{b}_{jc}_{jt}_{g}",
                                             tag="mm512", bufs=2)
                                for u in range(4):
                                    t = g * 4 + u
                                    nc.tensor.transpose(
                                        tp[:, u * 128:(u + 1) * 128],
                                        qn_tiles[t][:, jt * 128:(jt + 1) * 128],
                                        ident[:],
                                    )
                                nc.vector.tensor_copy(
                                    dst[jg][:, g * 512:(g + 1) * 512], tp[:])

                # ================= Phase 2: attention =====================
                outT = [sb.tile([128, N], BF16, name=f"outT_b{b}_{d}", tag="work2k",
                                bufs=10) for d in range(ND)]
                for hp in range(HP):
                    pv = [ps.tile([HD + 1, 512], F32, name=f"pv_b{b}_{hp}_{i}",
                                  tag="pv", bufs=4) for i in range(4)]  # A0 A1 B0 B1
                    for m in range(NT):
                        st = ps.tile([128, N], F32, name=f"st_b{b}_{hp}_{m}",
                                     tag="st", bufs=1)
                        pt = sb.tile([128, N], BF16, name=f"pt_b{b}_{hp}_{m}",
                                     tag="work2k", bufs=10)
                        for half in range(2):  # head A (rows 0-63), head B (64-127)
                            r0, r1 = half * 64, half * 64 + 64
                            for nch in range(2):
                                nc.tensor.matmul(
                                    st[:, nch * 512:(nch + 1) * 512],
                                    kT[hp][r0:r1, m * 128:(m + 1) * 128],
                                    qT[hp][r0:r1, nch * 512:(nch + 1) * 512],
                                )
                            nc.scalar.activation(pt[:], st[:], EXP, scale=float(SCALE))
                            h = hp * 2 + half
                            for nch in range(2):
                                nc.tensor.matmul(
                                    pv[half * 2 + nch][:],
                                    vsb[m][:, h * (HD + 1):(h + 1) * (HD + 1)],
                                    pt[:, nch * 512:(nch + 1) * 512],
                                    start=(m == 0), stop=(m == NT - 1),
                                )
                    # normalize: outT[h] = pv[0:64] * (1/denom) ; denom = pv row 64
                    for half in range(2):
                        h = hp * 2 + half
                        for nch in range(2):
                            p = pv[half * 2 + nch]
                            dr = sb.tile([1, 512], F32, name=f"dr_b{b}_{h}_{nch}",
                                         tag="dr", bufs=2)
                            nc.vector.tensor_copy(dr[:], p[64:65, :])
                            rr = sb.tile([1, 512], F32, name=f"rr_b{b}_{h}_{nch}",
                                         tag="rr", bufs=2)
                            nc.vector.reciprocal_approx_fast(rr[:], dr[:])
                            rb = sb.tile([64, 512], F32, name=f"rb_b{b}_{h}_{nch}",
                                         tag="rb", bufs=2)
                            nc.gpsimd.partition_broadcast(rb[:], rr[:])
                            nc.vector.tensor_tensor(
                                outT[h // 2][(h % 2) * 64:(h % 2) * 64 + 64,
                                             nch * 512:(nch + 1) * 512],
                                p[0:64, :], rb[:], mybir.AluOpType.mult)

                # ================= Phase 3: output projection ==============
                for t in range(NT):
                    for ec in range(2):
                        py = ps.tile([128, 512], F32, name=f"py_b{b}_{t}_{ec}",
                                     tag="mm512", bufs=2)
                        for d in range(ND):
                            nc.tensor.matmul(
                                py[:],
                                outT[d][:, t * 128:(t + 1) * 128],
                                wprojT[d][:, ec * 512:(ec + 1) * 512],
                                start=(d == 0), stop=False,
                            )
                        nc.tensor.matmul(
                            py[:], ones_r[:], bproj[:, ec * 512:(ec + 1) * 512],
                            start=False, stop=True,
                        )
                        ysb = sb.tile([128, 512], F32, name=f"y_b{b}_{t}_{ec}",
                                      tag="work2k", bufs=10)
                        nc.vector.tensor_copy(ysb[:], py[:])
                        nc.sync.dma_start(
                            y_d.ap()[b, t * 128:(t + 1) * 128,
                                     ec * 512:(ec + 1) * 512],
                            ysb[:],
                        )

    nc.compile()
    return nc


_NC_CACHE = None


def kernel(x, w_qkv, w_proj, b_proj):
    global _NC_CACHE, last_exec_time_ns
    x = np.ascontiguousarray(np.asarray(x, np.float32))
    w_qkv = np.asarray(w_qkv, np.float32)
    w_proj = np.asarray(w_proj, np.float32)
    b_proj = np.asarray(b_proj, np.float32)

    if _NC_CACHE is None:
        _NC_CACHE = _build()
    nc = _NC_CACHE

    cos_h, sin_h = _freq_tables()
    wqkvT = _round13(np.ascontiguousarray(w_qkv.T))
    import ml_dtypes
    wprojT16 = np.ascontiguousarray(w_proj.T).astype(ml_dtypes.bfloat16)
    bproj16 = b_proj.reshape(1, DIM).astype(ml_dtypes.bfloat16)
    ones16 = np.ones((1, 128), ml_dtypes.bfloat16)
    ident = np.eye(128, dtype=np.float32)

    in_maps = []
    for c in range(NCORES):
        xs = x[c * BPC:(c + 1) * BPC]                       # [2, N, DIM]
        xT = _round13(np.ascontiguousarray(xs.transpose(0, 2, 1)))
        in_maps.append({
            "xT": xT, "wqkvT": wqkvT, "wprojT": wprojT16,
            "bproj": bproj16, "cosh": cos_h, "sinh": sin_h,
            "ident": ident, "ones": ones16,
        })

    trace = bool(os.environ.get("KERNEL_TRACE"))
    kwargs = {}
    if trace:
        kwargs["trace"] = True
        td = os.environ.get("KERNEL_TRACE_DIR")
        if td:
            kwargs["tmpdir"] = td
    res = bass_utils.run_bass_kernel_spmd(
        nc, in_maps, core_ids=list(range(NCORES)), **kwargs)
    last_exec_time_ns = res.exec_time_ns
    out = np.concatenate([res.results[c]["y"] for c in range(NCORES)], axis=0)
    return np.ascontiguousarray(out.reshape(B, N, DIM).astype(np.float32))


if __name__ == "__main__":
    rng = np.random.default_rng(0)
    xs = rng.standard_normal((B, N, DIM), dtype=np.float32)
    wq = rng.standard_normal((3 * DIM, DIM), dtype=np.float32) / 32
    wp = rng.standard_normal((DIM, DIM), dtype=np.float32) / 32
    bp = np.zeros(DIM, np.float32)
    y = kernel(xs, wq, wp, bp)
    print("y", y.shape, y.dtype, float(np.abs(y).max()))
